# revision 43
# baseline (speedup 1.0000x reference)
"""Self-contained 8-core Trainium2 Bass kernel for MultiHeadAttention.

Problem: B=2, S=2048, D=1024, H=16 heads (hd=64), f32, self-attention
(no mask), eval mode (dropout = identity).

Sharding: data-parallel over B (2) x tensor-parallel over heads (4 groups
of 4 heads) = 8 cores. Each core computes, for its batch b and its 4
heads: Q/K/V projections (column-sliced), attention, and a partial
output projection (row-sliced Wo). Host sums the 4 partials per batch
and adds the (bv @ Wo + bo) correction (bv never enters the kernel:
ctx rows sum probs to 1, so (ctx+bv) @ Wo = ctx @ Wo + bv @ Wo).

Algebraic simplifications used (exact):
  - bk dropped: softmax over k is invariant to the per-q constant Q.bk.
  - softmax computed without max subtraction (scores bounded ~|s|<10,
    exp is safe in f32).
  - bq folded into Q^T as a per-partition bias.
  - row normalization deferred past the P@V matmul (scale ctx instead
    of probs); row sums obtained free via an appended ones-column in V.

Performance design (v3):
  - all matmul operands bf16 (f32r streams at 0.5 col/cycle, bf16 at
    1 col/cycle); PSUM accumulation stays f32. Softmax numerator and
    denominator share the bf16 exp values so normalization error
    largely cancels.
  - scores per head-pair as two concurrent K=64 row-tiled matmuls
    (tile_position (0,0)/(64,0)).
  - ACT exp (128 x [128,1024] tiles ~ 142us) is the bottleneck, so the
    emission is software-pipelined around the scores->exp stream: PV
    matmuls, projections, normalizations and the output projection are
    drained from a pending queue in the PE-slack of each exp step.
  - PSUM budget (8 banks): sreg [128,1024] x2 = 4, ctx/outproj shared
    ring [128,512] x3 = 3, proj ring [128,512] x1 = 1.
  - host pre-arranges xt/weights so every DMA is a contiguous
    per-partition block (9 DMAs total); PE warmup matmuls + ACT table
    preload run during the DMA fill.
"""

import sys

sys.path.insert(0, "/opt/trn_rl_repo")

import numpy as np

B, S, D, H, HD = 2, 2048, 1024, 16, 64
HPC = 4  # heads per core
NCORES = 8
DC = D // 128  # 8 contraction chunks
ST = S // 128  # 16 s-tiles
QCW = 512  # q chunk width
QC = S // QCW  # 4 q chunks
KT = S // 128  # 16 k tiles

_CACHE = {}


def _build(repeat=1, ep_bufs=34, target=1.18):
    from collections import deque

    import concourse.bass as bass  # noqa: F401
    import concourse.mybir as mybir
    import concourse.tile as tile
    from concourse import bacc
    from concourse.library_config import attn as attn_lib

    F32 = mybir.dt.float32
    BF16 = mybir.dt.bfloat16
    AF = mybir.ActivationFunctionType

    nc = bacc.Bacc("TRN2", target_bir_lowering=False, debug=False)

    # host pre-arranged inputs (see _make_in_maps), one tensor per DMA,
    # ordered by when the pipeline first needs them (bq rides with wq)
    WSZ = DC * HPC * HD  # 2048 elems/partition per weight matrix
    XSZ = DC * QCW       # 4096 elems/partition per xt quarter
    wk_d = nc.dram_tensor("wka", [128, WSZ], BF16, kind="ExternalInput")
    xq_d = [nc.dram_tensor(f"xtq{i}", [128, XSZ], BF16, kind="ExternalInput")
            for i in range(QC)]
    wqb_d = nc.dram_tensor("wqb", [128, WSZ + 2], BF16, kind="ExternalInput")
    wv_d = nc.dram_tensor("wva", [128, WSZ], BF16, kind="ExternalInput")
    wo_d = nc.dram_tensor("woa", [128, WSZ], BF16, kind="ExternalInput")
    out_d = nc.dram_tensor("out_p", [S, D], F32, kind="ExternalOutput")

    with tile.TileContext(nc) as tc:
        nc.gpsimd.load_library(attn_lib)
        with (
            tc.tile_pool(name="wp", bufs=1) as wp,
            tc.tile_pool(name="xp", bufs=1) as xp,
            tc.tile_pool(name="qk", bufs=1) as qk,
            tc.tile_pool(name="vp", bufs=1) as vp,
            tc.tile_pool(name="ep", bufs=ep_bufs) as ep,
            tc.tile_pool(name="cp", bufs=1) as cp,
            tc.tile_pool(name="mp", bufs=2) as mp,
            tc.tile_pool(name="op", bufs=3) as op,
            tc.tile_pool(name="pp", bufs=2, space="PSUM") as pp,
        ):
            ones_f = wp.tile([128, 64], BF16, tag="onesf")
            nc.vector.memset(ones_f[:], 1.0)
            ones_w = wp.tile([128, 512], BF16, tag="onesw")
            nc.vector.memset(ones_w[:], 1.0)
            scrap = wp.tile([128, 8], BF16, tag="scrap")

            # ---- loads: fine-grained DMAs in just-in-time order
            wk_f = wp.tile([128, WSZ], BF16, tag="wk")
            nc.sync.dma_start(wk_f[:], wk_d[:])
            xq_f = [xp.tile([128, XSZ], BF16, tag=f"xq{i}", name=f"xq{i}")
                    for i in range(QC)]
            nc.sync.dma_start(xq_f[0][:], xq_d[0][:])
            wqb_f = wp.tile([128, WSZ + 2], BF16, tag="wqb")
            nc.sync.dma_start(wqb_f[:], wqb_d[:])
            wv_f = wp.tile([128, WSZ], BF16, tag="wv")
            nc.sync.dma_start(wv_f[:], wv_d[:])
            for i in range(1, QC):
                nc.sync.dma_start(xq_f[i][:], xq_d[i][:])
            wo_f = wp.tile([128, WSZ], BF16, tag="wo")
            nc.sync.dma_start(wo_f[:], wo_d[:])

            wk_t = wk_f.rearrange("p (c n) -> p c n", n=HPC * HD)
            wq_t = wqb_f[:, 0:WSZ].rearrange("p (c n) -> p c n", n=HPC * HD)
            bq_b = wqb_f[:, WSZ:WSZ + 2]
            bq_t = wp.tile([128, 2], F32, tag="bq")
            nc.vector.tensor_copy(bq_t[:], bq_b)
            wv_t = wv_f.rearrange("p (c n) -> p c n", n=HPC * HD)
            wo_t = wo_f.rearrange("p (g n) -> p g n", g=2)
            _xtv = [x.rearrange("p (c w) -> p c w", w=QCW) for x in xq_f]

            import contextlib
            if repeat > 1:
                _engs = [mybir.EngineType.PE, mybir.EngineType.Activation,
                         mybir.EngineType.DVE, mybir.EngineType.SP,
                         mybir.EngineType.Pool]
                rep_ctx = tc.For_i(0, repeat, hint_engines=_engs, staggered_reset=True)
            else:
                rep_ctx = contextlib.nullcontext()
            with rep_ctx:
                # ---- ACT exp-table preload + PE HAM warmup during DMA fill
                nc.scalar.activation(scrap[:, 0:8], ones_f[:, 0:8], AF.Exp)
                warm_ps = pp.tile([64, 512], F32, tag="qkv", bufs=1, name="warm")
                for _w in range(18):
                    nc.tensor.matmul(warm_ps[:], ones_f[:, 0:64], ones_w[:],
                                     start=True, stop=True)

                # ---- V accumulator [s, 4*(64+1)] with ones columns
                v1_t = vp.tile([128, ST, HPC * 65], BF16, tag="v1")
                nc.vector.memset(
                    v1_t[:].rearrange("p s (h c) -> p s h c", c=65)[:, :, :, 64], 1.0)

                def v_proj(st):
                    vps = pp.tile([128, HPC * HD], F32, tag="qkv", bufs=1, name="vps")
                    for c in range(DC):
                        nc.tensor.matmul(
                            vps[:],
                            _xtv[st // 4][:, c, (st % 4) * 128:(st % 4 + 1) * 128],
                            wv_t[:, c, :],
                            start=(c == 0),
                            stop=(c == DC - 1),
                        )
                    with nc.allow_low_precision(reason="bf16 matmul operands"):
                        nc.vector.tensor_copy(
                            v1_t[:, st, :].rearrange("p (h c) -> p h c", c=65)[:, :, 0:64],
                            vps[:].rearrange("p (h c) -> p h c", c=64),
                        )

                qt_tiles = [qk.tile([128, S], BF16, tag=f"qt{p}", name=f"qt{p}") for p in range(2)]
                kt_tiles = [qk.tile([128, S], BF16, tag=f"kt{p}", name=f"kt{p}") for p in range(2)]

                _proj_ps = {}

                def _proj(w_t, pair, qcc, cs):
                    """Half of a K/Q projection (contraction chunks cs);
                    both halves share one PSUM tile."""
                    key = (w_t.name, pair, qcc)
                    if key not in _proj_ps:
                        _proj_ps[key] = pp.tile([128, QCW], F32, tag="qkv",
                                                bufs=1, name="prps")
                    prps = _proj_ps[key]
                    for c in cs:
                        nc.tensor.matmul(
                            prps[:],
                            w_t[:, c, pair * 128:(pair + 1) * 128],
                            _xtv[qcc][:, c, :],
                            start=(c == 0),
                            stop=(c == DC - 1),
                        )
                    return prps

                def kt_proj(pair, qcc, cs=range(DC)):
                    kps = _proj(wk_t, pair, qcc, cs)
                    if cs[-1] == DC - 1:
                        qs = slice(qcc * QCW, (qcc + 1) * QCW)
                        with nc.allow_low_precision(reason="bf16 score operands"):
                            nc.vector.tensor_copy(kt_tiles[pair][:, qs], kps[:])

                def qt_proj(pair, qcc, cs=range(DC)):
                    qps = _proj(wq_t, pair, qcc, cs)
                    if cs[-1] == DC - 1:
                        qs = slice(qcc * QCW, (qcc + 1) * QCW)
                        with nc.allow_low_precision(reason="bf16 score operands"):
                            nc.vector.tensor_scalar_add(
                                qt_tiles[pair][:, qs], qps[:], bq_t[:, pair:pair + 1]
                            )

                ctxt_tiles = [cp.tile([128, S], BF16, tag=f"ct{p}", name=f"ct{p}") for p in range(2)]

                # ---- software pipeline ------------------------------------
                # pending: deque of (pe_cost_us, closure) drained in PE slack
                pending = deque()

                def drain(budget):
                    while pending and budget > 0.0:
                        cost, fn = pending.popleft()
                        fn()
                        budget -= cost
                    return budget

                ctx_ps = {}   # (pair, qcc) -> [h0_tile, h1_tile]
                expt_of = {}  # (pair, qcc, r) -> expt tile

                def scores_exp(pair, qcc, r):
                    qs = slice(qcc * QCW, (qcc + 1) * QCW)
                    sreg = pp.tile([128, 2 * QCW], F32, tag="big")
                    expt = ep.tile([128, 2 * QCW], BF16, tag="exp")
                    for h in range(2):
                        nc.tensor.matmul(
                            sreg[:, h * QCW:(h + 1) * QCW],
                            kt_tiles[pair][64 * h:64 * (h + 1), r * 128:(r + 1) * 128],
                            qt_tiles[pair][64 * h:64 * (h + 1), qs],
                            start=True,
                            stop=True,
                            tile_position=(64 * h, 0),
                        )
                    with nc.allow_low_precision(reason="bf16 probs"):
                        nc.scalar.activation(expt[:], sreg[:], AF.Exp, scale=0.125)
                    expt_of[(pair, qcc, r)] = expt

                def pv(pair, qcc, h, r):
                    key = (pair, qcc)
                    if key not in ctx_ps:
                        ctx_ps[key] = [
                            pp.tile([65, QCW], F32, tag="ctx", bufs=3,
                                    name=f"ctx{pair}{qcc}{_h}")
                            for _h in range(2)
                        ]
                    hh = 2 * pair + h
                    expt = expt_of[(pair, qcc, r)]
                    nc.tensor.matmul(
                        ctx_ps[key][h][:],
                        v1_t[:, r, 65 * hh:65 * hh + 65],
                        expt[:, h * QCW:(h + 1) * QCW],
                        start=(r == 0),
                        stop=(r == KT - 1),
                    )
                    if h == 1:  # h1 trails h0, so it is the last reader
                        expt_of.pop((pair, qcc, r), None)

                def norm(pair, qcc, h, sub=0, w=QCW):
                    # normalize a w-wide slice (sub indexes units of w)
                    o = sub * w
                    qs = slice(qcc * QCW + o, qcc * QCW + o + w)
                    cps = ctx_ps[(pair, qcc)][h]
                    rsum = mp.tile([1, QCW], F32, tag="rsum")
                    nc.vector.reciprocal(rsum[:, 0:w], cps[64:65, o:o + w])
                    bct = mp.tile([64, QCW], F32, tag="bc")
                    nc.gpsimd.partition_broadcast(bct[:, 0:w], rsum[:, 0:w])
                    with nc.allow_low_precision(reason="bf16 matmul operands"):
                        nc.vector.tensor_mul(
                            ctxt_tiles[pair][64 * h:64 * (h + 1), qs],
                            cps[0:64, o:o + w],
                            bct[:, 0:w],
                        )

                osb_of = {}

                def outproj_block(qcc, sub, d2):
                    q0 = qcc * QCW + sub * 128
                    ops = pp.tile([128, 512], F32, tag="ctx", bufs=3, name="ops")
                    for pair in range(2):
                        nc.tensor.matmul(
                            ops[:],
                            ctxt_tiles[pair][:, q0:q0 + 128],
                            wo_t[:, pair, d2 * 512:(d2 + 1) * 512],
                            start=(pair == 0),
                            stop=(pair == 1),
                        )
                    key = (qcc, sub)
                    if key not in osb_of:
                        osb_of[key] = op.tile([128, 2, 512], F32, tag="osb",
                                              name="osb")
                    osb = osb_of.pop(key) if d2 == 1 else osb_of[key]
                    nc.vector.tensor_copy(osb[:, d2, :], ops[:])
                    if d2 == 1:
                        # one 4KB-per-partition DMA per 128-row output block
                        nc.sync.dma_start(
                            out_d[q0:q0 + 128, :],
                            osb[:].rearrange("p a b -> p (a b)"))

                def push_loop_work(pair, qcc):
                    """Queue all h0 PVs (h0 norms pop mid-next-loop, off the
                    critical path), then h1 PVs with norms and (pair 1) the
                    output projection interleaved per 128-sub at the end so
                    only the short h1 chain sits at the loop boundary."""
                    items = []
                    last = (pair == 1 and qcc == QC - 1)
                    if last:
                        # self-PVs for r < KT-2 were pushed inline during the
                        # loop; finish h0 first so its norms start on DVE
                        # while PE runs the remaining h1 PVs
                        for r in range(KT - 2, KT):
                            items.append((0.22, (lambda r=r: pv(1, QC - 1, 0, r))))
                        for s in range(4):
                            items.append((0.05, (lambda s=s: norm(1, QC - 1, 0, s, 128))))
                        for r in range(KT - 2, KT):
                            items.append((0.22, (lambda r=r: pv(1, QC - 1, 1, r))))
                        pending.extend(items)
                        return
                    for r in range(KT):
                        items.append((0.22, (lambda p=pair, q=qcc, r=r: pv(p, q, 0, r))))
                    if True:
                        if pair == 0:
                            items.append((0.05, (lambda q=qcc: norm(0, q, 0, 0, 256))))
                            items.append((0.05, (lambda q=qcc: norm(0, q, 0, 1, 256))))
                        else:
                            for s in range(4):
                                items.append((0.05, (lambda q=qcc, s=s:
                                                     norm(1, q, 0, s, 128))))
                    for r in range(KT):
                        items.append((0.22, (lambda p=pair, q=qcc, r=r: pv(p, q, 1, r))))
                    if not last:
                        if pair == 0:
                            items.append((0.05, (lambda q=qcc: norm(0, q, 1, 0, 256))))
                            items.append((0.05, (lambda q=qcc: norm(0, q, 1, 1, 256))))
                        else:
                            for s in range(4):
                                items.append((0.05, (lambda q=qcc, s=s:
                                                     norm(1, q, 1, s, 128))))
                                items.append((0.45, (lambda q=qcc, s=s:
                                                     outproj_block(q, s, 0))))
                                items.append((0.45, (lambda q=qcc, s=s:
                                                     outproj_block(q, s, 1))))
                    pending.extend(items)

                # hard injections: (loop_index, r) -> list of (cost, fn)
                hard = {}

                def add_hard(li, r, cost, fn):
                    hard.setdefault((li, r), []).append((cost, fn))

                H1, H2 = range(0, DC // 2), range(DC // 2, DC)

                # qt for next qc of same pair, split in two halves
                for li, (pair, qcc) in enumerate(
                        [(p, q) for p in range(2) for q in range(QC)]):
                    if qcc < QC - 1:
                        add_hard(li, 9, 0.85, (lambda p=pair, q=qcc + 1: qt_proj(p, q, H1)))
                        add_hard(li, 11, 0.85, (lambda p=pair, q=qcc + 1: qt_proj(p, q, H2)))
                # kt/qt for pair 1 spread over pair-0 loops 2,3
                add_hard(2, 2, 0.85, lambda: kt_proj(1, 0, H1))
                add_hard(2, 4, 0.85, lambda: kt_proj(1, 0, H2))
                add_hard(2, 6, 0.85, lambda: kt_proj(1, 1, H1))
                add_hard(2, 8, 0.85, lambda: kt_proj(1, 1, H2))
                add_hard(3, 2, 0.85, lambda: kt_proj(1, 2, H1))
                add_hard(3, 4, 0.85, lambda: kt_proj(1, 2, H2))
                add_hard(3, 6, 0.85, lambda: kt_proj(1, 3, H1))
                add_hard(3, 8, 0.85, lambda: kt_proj(1, 3, H2))
                add_hard(3, 13, 0.85, lambda: qt_proj(1, 0, H1))
                add_hard(3, 14, 0.85, lambda: qt_proj(1, 0, H2))
                # loop 0: V projection + JIT kt(0,1..3) as the xt DMA lands
                vq = 0
                for r in range(KT):
                    if r in (3, 7, 11):
                        add_hard(0, r, 1.7, (lambda q=r // 4 + 1: kt_proj(0, q)))
                    else:
                        add_hard(0, r, 0.86, (lambda st=vq: v_proj(st)))
                        vq += 1
                for j in range(3):
                    add_hard(1, j, 0.86, (lambda st=13 + j: v_proj(st)))

                # ---- prelude
                kt_proj(0, 0)
                qt_proj(0, 0)

                # ---- main loops
                for li, (pair, qcc) in enumerate(
                        [(p, q) for p in range(2) for q in range(QC)]):
                    for r in range(KT):
                        budget = target - 0.21
                        for cost, fn in hard.pop((li, r), []):
                            fn()
                            budget -= cost
                        scores_exp(pair, qcc, r)
                        if li == 7 and r >= 2:
                            # last loop: self-PVs join the queue right away
                            # so they drain in-loop instead of in the tail
                            pending.append((0.22, (lambda r=r - 2: pv(1, QC - 1, 0, r))))
                            pending.append((0.22, (lambda r=r - 2: pv(1, QC - 1, 1, r))))
                        drain(budget)
                    push_loop_work(pair, qcc)

                # ---- drain tail; last loop's h1 norms split per 128-wide
                # sub-chunk, pipelined with its output projection
                while pending:
                    _, fn = pending.popleft()
                    fn()
                for sub in range(4):
                    norm(1, QC - 1, 1, sub=sub, w=128)
                    outproj_block(QC - 1, sub, 0)
                    outproj_block(QC - 1, sub, 1)

    nc.compile()
    return nc


def _get_nc(repeat=1):
    key = repeat
    if key not in _CACHE:
        _CACHE[key] = _build(repeat)
    return _CACHE[key]


def _part_major_flat(a):
    """[G*128, N] -> [128, G*N] (partition-major, flattened)."""
    n = a.shape[1]
    return a.reshape(-1, 128, n).transpose(1, 0, 2).reshape(128, -1)


def _make_in_maps(query_input, Wq, bq, Wk, Wv, Wo):
    import ml_dtypes

    BF = ml_dtypes.bfloat16
    x = np.asarray(query_input, dtype=np.float32)
    in_maps = []
    for core in range(NCORES):
        b, g = divmod(core, NCORES // B)
        cs = slice(g * HPC * HD, (g + 1) * HPC * HD)
        xt = x[b].T.astype(BF)  # [D, S]
        # [D, S] -> [128, QC, DC*QCW]: partition p, quarter qc, chunk c
        xtq = xt.reshape(DC, 128, QC, QCW).transpose(1, 2, 0, 3).reshape(128, QC, -1)
        wqa = _part_major_flat(Wq[:, cs].astype(BF))
        bq2 = bq[cs].reshape(2, 128).T.astype(BF)
        m = {
            "wka": _part_major_flat(Wk[:, cs].astype(BF)),
            "wqb": np.concatenate([wqa, bq2], axis=1),
            "wva": _part_major_flat(Wv[:, cs].astype(BF)),
            "woa": _part_major_flat(Wo[cs, :].astype(BF)),
        }
        for i in range(QC):
            m[f"xtq{i}"] = xtq[:, i]
        in_maps.append({k: np.ascontiguousarray(v) for k, v in m.items()})
    return in_maps


def kernel(query_input, Wq, bq, Wk, bk, Wv, bv, Wo, bo):
    from concourse.bass_utils import run_bass_kernel_spmd

    Wq = np.asarray(Wq, np.float32)
    Wk = np.asarray(Wk, np.float32)
    Wv = np.asarray(Wv, np.float32)
    Wo = np.asarray(Wo, np.float32)
    bq = np.asarray(bq, np.float32)
    bv = np.asarray(bv, np.float32)
    bo = np.asarray(bo, np.float32)

    nc = _get_nc()
    in_maps = _make_in_maps(query_input, Wq, bq, Wk, Wv, Wo)
    res = run_bass_kernel_spmd(nc, in_maps, core_ids=list(range(NCORES)))

    gpc = NCORES // B  # groups per batch
    out = np.zeros((B, S, D), np.float32)
    for core in range(NCORES):
        b = core // gpc
        out[b] += res.results[core]["out_p"]
    # bv correction (exact) + bo, applied once on the full output
    out += (bv @ Wo + bo)[None, None, :]
    return out


# revision 44
# speedup vs baseline: 1.0309x; 1.0309x over previous
"""Self-contained 8-core Trainium2 Bass kernel for MultiHeadAttention.

Problem: B=2, S=2048, D=1024, H=16 heads (hd=64), f32, self-attention
(no mask), eval mode (dropout = identity).

Sharding: data-parallel over B (2) x tensor-parallel over heads (4 groups
of 4 heads) = 8 cores. Each core computes, for its batch b and its 4
heads: Q/K/V projections (column-sliced), attention, and a partial
output projection (row-sliced Wo). Host sums the 4 partials per batch
and adds the (bv @ Wo + bo) correction (bv never enters the kernel:
ctx rows sum probs to 1, so (ctx+bv) @ Wo = ctx @ Wo + bv @ Wo).

Algebraic simplifications used (exact):
  - bk dropped: softmax over k is invariant to the per-q constant Q.bk.
  - softmax computed without max subtraction (scores bounded ~|s|<10,
    exp is safe in f32).
  - bq folded into Q^T as a per-partition bias.
  - row normalization deferred past the P@V matmul (scale ctx instead
    of probs); row sums obtained free via an appended ones-column in V.

Performance design (v3):
  - all matmul operands bf16 (f32r streams at 0.5 col/cycle, bf16 at
    1 col/cycle); PSUM accumulation stays f32. Softmax numerator and
    denominator share the bf16 exp values so normalization error
    largely cancels.
  - scores per head-pair as two concurrent K=64 row-tiled matmuls
    (tile_position (0,0)/(64,0)).
  - ACT exp (128 x [128,1024] tiles ~ 142us) is the bottleneck, so the
    emission is software-pipelined around the scores->exp stream: PV
    matmuls, projections, normalizations and the output projection are
    drained from a pending queue in the PE-slack of each exp step.
  - PSUM budget (8 banks): sreg [128,1024] x2 = 4, ctx/outproj shared
    ring [128,512] x3 = 3, proj ring [128,512] x1 = 1.
  - host pre-arranges xt/weights so every DMA is a contiguous
    per-partition block (9 DMAs total); PE warmup matmuls + ACT table
    preload run during the DMA fill.
"""

import sys

sys.path.insert(0, "/opt/trn_rl_repo")

import numpy as np

B, S, D, H, HD = 2, 2048, 1024, 16, 64
HPC = 4  # heads per core
NCORES = 8
DC = D // 128  # 8 contraction chunks
ST = S // 128  # 16 s-tiles
QCW = 512  # q chunk width
QC = S // QCW  # 4 q chunks
KT = S // 128  # 16 k tiles

_CACHE = {}


def _build(repeat=1, ep_bufs=34, target=1.08):
    from collections import deque

    import concourse.bass as bass  # noqa: F401
    import concourse.mybir as mybir
    import concourse.tile as tile
    from concourse import bacc
    from concourse.library_config import attn as attn_lib

    F32 = mybir.dt.float32
    BF16 = mybir.dt.bfloat16
    AF = mybir.ActivationFunctionType

    nc = bacc.Bacc("TRN2", target_bir_lowering=False, debug=False)

    # host pre-arranged inputs (see _make_in_maps), one tensor per DMA,
    # ordered by when the pipeline first needs them (bq rides with wq)
    WSZ = DC * HPC * HD  # 2048 elems/partition per weight matrix
    XSZ = DC * QCW       # 4096 elems/partition per xt quarter
    wk_d = nc.dram_tensor("wka", [128, WSZ], BF16, kind="ExternalInput")
    xq_d = [nc.dram_tensor(f"xtq{i}", [128, XSZ], BF16, kind="ExternalInput")
            for i in range(QC)]
    wqb_d = nc.dram_tensor("wqb", [128, WSZ + 2], BF16, kind="ExternalInput")
    wv_d = nc.dram_tensor("wva", [128, WSZ], BF16, kind="ExternalInput")
    wo_d = nc.dram_tensor("woa", [128, WSZ], BF16, kind="ExternalInput")
    out_d = nc.dram_tensor("out_p", [S, D], F32, kind="ExternalOutput")

    with tile.TileContext(nc) as tc:
        nc.gpsimd.load_library(attn_lib)
        with (
            tc.tile_pool(name="wp", bufs=1) as wp,
            tc.tile_pool(name="xp", bufs=1) as xp,
            tc.tile_pool(name="qk", bufs=1) as qk,
            tc.tile_pool(name="vp", bufs=1) as vp,
            tc.tile_pool(name="ep", bufs=ep_bufs) as ep,
            tc.tile_pool(name="cp", bufs=1) as cp,
            tc.tile_pool(name="mp", bufs=2) as mp,
            tc.tile_pool(name="op", bufs=3) as op,
            tc.tile_pool(name="pp", bufs=2, space="PSUM") as pp,
        ):
            ones_f = wp.tile([128, 64], BF16, tag="onesf")
            nc.vector.memset(ones_f[:], 1.0)
            ones_w = wp.tile([128, 512], BF16, tag="onesw")
            nc.vector.memset(ones_w[:], 1.0)
            scrap = wp.tile([128, 8], BF16, tag="scrap")

            # ---- loads: fine-grained DMAs in just-in-time order
            wk_f = wp.tile([128, WSZ], BF16, tag="wk")
            nc.sync.dma_start(wk_f[:], wk_d[:])
            xq_f = [xp.tile([128, XSZ], BF16, tag=f"xq{i}", name=f"xq{i}")
                    for i in range(QC)]
            nc.sync.dma_start(xq_f[0][:], xq_d[0][:])
            wqb_f = wp.tile([128, WSZ + 2], BF16, tag="wqb")
            nc.sync.dma_start(wqb_f[:], wqb_d[:])
            wv_f = wp.tile([128, WSZ], BF16, tag="wv")
            nc.sync.dma_start(wv_f[:], wv_d[:])
            for i in range(1, QC):
                nc.sync.dma_start(xq_f[i][:], xq_d[i][:])
            wo_f = wp.tile([128, WSZ], BF16, tag="wo")
            nc.sync.dma_start(wo_f[:], wo_d[:])

            wk_t = wk_f.rearrange("p (c n) -> p c n", n=HPC * HD)
            wq_t = wqb_f[:, 0:WSZ].rearrange("p (c n) -> p c n", n=HPC * HD)
            bq_b = wqb_f[:, WSZ:WSZ + 2]
            bq_t = wp.tile([128, 2], F32, tag="bq")
            nc.vector.tensor_copy(bq_t[:], bq_b)
            wv_t = wv_f.rearrange("p (c n) -> p c n", n=HPC * HD)
            wo_t = wo_f.rearrange("p (g n) -> p g n", g=2)
            _xtv = [x.rearrange("p (c w) -> p c w", w=QCW) for x in xq_f]

            import contextlib
            if repeat > 1:
                _engs = [mybir.EngineType.PE, mybir.EngineType.Activation,
                         mybir.EngineType.DVE, mybir.EngineType.SP,
                         mybir.EngineType.Pool]
                rep_ctx = tc.For_i(0, repeat, hint_engines=_engs, staggered_reset=True)
            else:
                rep_ctx = contextlib.nullcontext()
            with rep_ctx:
                # ---- ACT exp-table preload + PE HAM warmup during DMA fill
                nc.scalar.activation(scrap[:, 0:8], ones_f[:, 0:8], AF.Exp)
                warm_ps = pp.tile([64, 512], F32, tag="qkv", bufs=1, name="warm")
                for _w in range(18):
                    nc.tensor.matmul(warm_ps[:], ones_f[:, 0:64], ones_w[:],
                                     start=True, stop=True)

                # ---- V accumulator [s, 4*(64+1)] with ones columns
                v1_t = vp.tile([128, ST, HPC * 65], BF16, tag="v1")
                nc.vector.memset(
                    v1_t[:].rearrange("p s (h c) -> p s h c", c=65)[:, :, :, 64], 1.0)

                def v_proj(st):
                    vps = pp.tile([128, HPC * HD], F32, tag="qkv", bufs=1, name="vps")
                    for c in range(DC):
                        nc.tensor.matmul(
                            vps[:],
                            _xtv[st // 4][:, c, (st % 4) * 128:(st % 4 + 1) * 128],
                            wv_t[:, c, :],
                            start=(c == 0),
                            stop=(c == DC - 1),
                        )
                    with nc.allow_low_precision(reason="bf16 matmul operands"):
                        nc.vector.tensor_copy(
                            v1_t[:, st, :].rearrange("p (h c) -> p h c", c=65)[:, :, 0:64],
                            vps[:].rearrange("p (h c) -> p h c", c=64),
                        )

                qt_tiles = [qk.tile([128, S], BF16, tag=f"qt{p}", name=f"qt{p}") for p in range(2)]
                kt_tiles = [qk.tile([128, S], BF16, tag=f"kt{p}", name=f"kt{p}") for p in range(2)]

                _proj_ps = {}

                def _proj(w_t, pair, qcc, cs):
                    """Half of a K/Q projection (contraction chunks cs);
                    both halves share one PSUM tile."""
                    key = (w_t.name, pair, qcc)
                    if key not in _proj_ps:
                        _proj_ps[key] = pp.tile([128, QCW], F32, tag="qkv",
                                                bufs=1, name="prps")
                    prps = _proj_ps[key]
                    for c in cs:
                        nc.tensor.matmul(
                            prps[:],
                            w_t[:, c, pair * 128:(pair + 1) * 128],
                            _xtv[qcc][:, c, :],
                            start=(c == 0),
                            stop=(c == DC - 1),
                        )
                    return prps

                def kt_proj(pair, qcc, cs=range(DC)):
                    kps = _proj(wk_t, pair, qcc, cs)
                    if cs[-1] == DC - 1:
                        qs = slice(qcc * QCW, (qcc + 1) * QCW)
                        with nc.allow_low_precision(reason="bf16 score operands"):
                            nc.vector.tensor_copy(kt_tiles[pair][:, qs], kps[:])

                def qt_proj(pair, qcc, cs=range(DC)):
                    qps = _proj(wq_t, pair, qcc, cs)
                    if cs[-1] == DC - 1:
                        qs = slice(qcc * QCW, (qcc + 1) * QCW)
                        with nc.allow_low_precision(reason="bf16 score operands"):
                            nc.vector.tensor_scalar_add(
                                qt_tiles[pair][:, qs], qps[:], bq_t[:, pair:pair + 1]
                            )

                ctxt_tiles = [cp.tile([128, S], BF16, tag=f"ct{p}", name=f"ct{p}") for p in range(2)]

                # ---- software pipeline ------------------------------------
                # pending: deque of (pe_cost_us, closure) drained in PE slack
                pending = deque()

                def drain(budget):
                    while pending and budget > 0.0:
                        cost, fn = pending.popleft()
                        fn()
                        budget -= cost
                    return budget

                ctx_ps = {}   # (pair, qcc) -> [h0_tile, h1_tile]
                expt_of = {}  # (pair, qcc, r) -> expt tile

                def scores_exp(pair, qcc, r):
                    qs = slice(qcc * QCW, (qcc + 1) * QCW)
                    sreg = pp.tile([128, 2 * QCW], F32, tag="big")
                    expt = ep.tile([128, 2 * QCW], BF16, tag="exp")
                    for h in range(2):
                        nc.tensor.matmul(
                            sreg[:, h * QCW:(h + 1) * QCW],
                            kt_tiles[pair][64 * h:64 * (h + 1), r * 128:(r + 1) * 128],
                            qt_tiles[pair][64 * h:64 * (h + 1), qs],
                            start=True,
                            stop=True,
                            tile_position=(64 * h, 0),
                        )
                    with nc.allow_low_precision(reason="bf16 probs"):
                        nc.scalar.activation(expt[:], sreg[:], AF.Exp, scale=0.125)
                    expt_of[(pair, qcc, r)] = expt

                def pv(pair, qcc, h, r):
                    key = (pair, qcc)
                    if key not in ctx_ps:
                        ctx_ps[key] = [
                            pp.tile([65, QCW], F32, tag="ctx", bufs=3,
                                    name=f"ctx{pair}{qcc}{_h}")
                            for _h in range(2)
                        ]
                    hh = 2 * pair + h
                    expt = expt_of[(pair, qcc, r)]
                    nc.tensor.matmul(
                        ctx_ps[key][h][:],
                        v1_t[:, r, 65 * hh:65 * hh + 65],
                        expt[:, h * QCW:(h + 1) * QCW],
                        start=(r == 0),
                        stop=(r == KT - 1),
                    )
                    if h == 1:  # h1 trails h0, so it is the last reader
                        expt_of.pop((pair, qcc, r), None)

                def norm(pair, qcc, h, sub=0, w=QCW):
                    # normalize a w-wide slice (sub indexes units of w)
                    o = sub * w
                    qs = slice(qcc * QCW + o, qcc * QCW + o + w)
                    cps = ctx_ps[(pair, qcc)][h]
                    rsum = mp.tile([1, QCW], F32, tag="rsum")
                    nc.vector.reciprocal(rsum[:, 0:w], cps[64:65, o:o + w])
                    bct = mp.tile([64, QCW], F32, tag="bc")
                    nc.gpsimd.partition_broadcast(bct[:, 0:w], rsum[:, 0:w])
                    with nc.allow_low_precision(reason="bf16 matmul operands"):
                        nc.vector.tensor_mul(
                            ctxt_tiles[pair][64 * h:64 * (h + 1), qs],
                            cps[0:64, o:o + w],
                            bct[:, 0:w],
                        )

                osb_of = {}

                def outproj_block(qcc, sub, d2):
                    q0 = qcc * QCW + sub * 128
                    ops = pp.tile([128, 512], F32, tag="ctx", bufs=3, name="ops")
                    for pair in range(2):
                        nc.tensor.matmul(
                            ops[:],
                            ctxt_tiles[pair][:, q0:q0 + 128],
                            wo_t[:, pair, d2 * 512:(d2 + 1) * 512],
                            start=(pair == 0),
                            stop=(pair == 1),
                        )
                    key = (qcc, sub)
                    if key not in osb_of:
                        osb_of[key] = op.tile([128, 2, 512], F32, tag="osb",
                                              name="osb")
                    osb = osb_of.pop(key) if d2 == 1 else osb_of[key]
                    nc.vector.tensor_copy(osb[:, d2, :], ops[:])
                    if d2 == 1:
                        # one 4KB-per-partition DMA per 128-row output block
                        nc.sync.dma_start(
                            out_d[q0:q0 + 128, :],
                            osb[:].rearrange("p a b -> p (a b)"))

                def push_loop_work(pair, qcc):
                    """Queue all h0 PVs (h0 norms pop mid-next-loop, off the
                    critical path), then h1 PVs with norms and (pair 1) the
                    output projection interleaved per 128-sub at the end so
                    only the short h1 chain sits at the loop boundary."""
                    items = []
                    last = (pair == 1 and qcc == QC - 1)
                    if last:
                        # self-PVs for r < KT-2 were pushed inline during the
                        # loop; finish h0 first so its norms start on DVE
                        # while PE runs the remaining h1 PVs
                        for r in range(KT - 2, KT):
                            items.append((0.22, (lambda r=r: pv(1, QC - 1, 0, r))))
                        for s in range(4):
                            items.append((0.05, (lambda s=s: norm(1, QC - 1, 0, s, 128))))
                        for r in range(KT - 2, KT):
                            items.append((0.22, (lambda r=r: pv(1, QC - 1, 1, r))))
                        pending.extend(items)
                        return
                    for r in range(KT):
                        items.append((0.22, (lambda p=pair, q=qcc, r=r: pv(p, q, 0, r))))
                    if True:
                        if pair == 0:
                            items.append((0.05, (lambda q=qcc: norm(0, q, 0, 0, 256))))
                            items.append((0.05, (lambda q=qcc: norm(0, q, 0, 1, 256))))
                        else:
                            for s in range(4):
                                items.append((0.05, (lambda q=qcc, s=s:
                                                     norm(1, q, 0, s, 128))))
                    for r in range(KT):
                        items.append((0.22, (lambda p=pair, q=qcc, r=r: pv(p, q, 1, r))))
                    if not last:
                        if pair == 0:
                            items.append((0.05, (lambda q=qcc: norm(0, q, 1, 0, 256))))
                            items.append((0.05, (lambda q=qcc: norm(0, q, 1, 1, 256))))
                        else:
                            for s in range(4):
                                items.append((0.05, (lambda q=qcc, s=s:
                                                     norm(1, q, 1, s, 128))))
                                items.append((0.45, (lambda q=qcc, s=s:
                                                     outproj_block(q, s, 0))))
                                items.append((0.45, (lambda q=qcc, s=s:
                                                     outproj_block(q, s, 1))))
                    pending.extend(items)

                # hard injections: (loop_index, r) -> list of (cost, fn)
                hard = {}

                def add_hard(li, r, cost, fn):
                    hard.setdefault((li, r), []).append((cost, fn))

                H1, H2 = range(0, DC // 2), range(DC // 2, DC)

                # qt for next qc of same pair, split in two halves
                for li, (pair, qcc) in enumerate(
                        [(p, q) for p in range(2) for q in range(QC)]):
                    if qcc < QC - 1:
                        add_hard(li, 9, 0.85, (lambda p=pair, q=qcc + 1: qt_proj(p, q, H1)))
                        add_hard(li, 11, 0.85, (lambda p=pair, q=qcc + 1: qt_proj(p, q, H2)))
                # kt/qt for pair 1 spread over pair-0 loops 2,3
                add_hard(2, 2, 0.85, lambda: kt_proj(1, 0, H1))
                add_hard(2, 4, 0.85, lambda: kt_proj(1, 0, H2))
                add_hard(2, 6, 0.85, lambda: kt_proj(1, 1, H1))
                add_hard(2, 8, 0.85, lambda: kt_proj(1, 1, H2))
                add_hard(3, 2, 0.85, lambda: kt_proj(1, 2, H1))
                add_hard(3, 4, 0.85, lambda: kt_proj(1, 2, H2))
                add_hard(3, 6, 0.85, lambda: kt_proj(1, 3, H1))
                add_hard(3, 8, 0.85, lambda: kt_proj(1, 3, H2))
                add_hard(3, 13, 0.85, lambda: qt_proj(1, 0, H1))
                add_hard(3, 14, 0.85, lambda: qt_proj(1, 0, H2))
                # loop 0: V projection + JIT kt(0,1..3) as the xt DMA lands
                vq = 0
                for r in range(KT):
                    if r in (3, 7, 11):
                        add_hard(0, r, 1.7, (lambda q=r // 4 + 1: kt_proj(0, q)))
                    else:
                        add_hard(0, r, 0.86, (lambda st=vq: v_proj(st)))
                        vq += 1
                for j in range(3):
                    add_hard(1, j, 0.86, (lambda st=13 + j: v_proj(st)))

                # ---- prelude
                kt_proj(0, 0)
                qt_proj(0, 0)

                # ---- main loops
                for li, (pair, qcc) in enumerate(
                        [(p, q) for p in range(2) for q in range(QC)]):
                    for r in range(KT):
                        budget = target - 0.21
                        for cost, fn in hard.pop((li, r), []):
                            fn()
                            budget -= cost
                        scores_exp(pair, qcc, r)
                        if li == 7 and r >= 2:
                            # last loop: self-PVs join the queue right away
                            # so they drain in-loop instead of in the tail
                            pending.append((0.22, (lambda r=r - 2: pv(1, QC - 1, 0, r))))
                            pending.append((0.22, (lambda r=r - 2: pv(1, QC - 1, 1, r))))
                        drain(budget)
                    push_loop_work(pair, qcc)

                # ---- drain tail; last loop's h1 norms split per 128-wide
                # sub-chunk, pipelined with its output projection
                while pending:
                    _, fn = pending.popleft()
                    fn()
                for sub in range(4):
                    norm(1, QC - 1, 1, sub=sub, w=128)
                    outproj_block(QC - 1, sub, 0)
                    outproj_block(QC - 1, sub, 1)

    nc.compile()
    return nc


def _get_nc(repeat=1):
    key = repeat
    if key not in _CACHE:
        _CACHE[key] = _build(repeat)
    return _CACHE[key]


def _part_major_flat(a):
    """[G*128, N] -> [128, G*N] (partition-major, flattened)."""
    n = a.shape[1]
    return a.reshape(-1, 128, n).transpose(1, 0, 2).reshape(128, -1)


def _make_in_maps(query_input, Wq, bq, Wk, Wv, Wo):
    import ml_dtypes

    BF = ml_dtypes.bfloat16
    x = np.asarray(query_input, dtype=np.float32)
    in_maps = []
    for core in range(NCORES):
        b, g = divmod(core, NCORES // B)
        cs = slice(g * HPC * HD, (g + 1) * HPC * HD)
        xt = x[b].T.astype(BF)  # [D, S]
        # [D, S] -> [128, QC, DC*QCW]: partition p, quarter qc, chunk c
        xtq = xt.reshape(DC, 128, QC, QCW).transpose(1, 2, 0, 3).reshape(128, QC, -1)
        wqa = _part_major_flat(Wq[:, cs].astype(BF))
        bq2 = bq[cs].reshape(2, 128).T.astype(BF)
        m = {
            "wka": _part_major_flat(Wk[:, cs].astype(BF)),
            "wqb": np.concatenate([wqa, bq2], axis=1),
            "wva": _part_major_flat(Wv[:, cs].astype(BF)),
            "woa": _part_major_flat(Wo[cs, :].astype(BF)),
        }
        for i in range(QC):
            m[f"xtq{i}"] = xtq[:, i]
        in_maps.append({k: np.ascontiguousarray(v) for k, v in m.items()})
    return in_maps


def kernel(query_input, Wq, bq, Wk, bk, Wv, bv, Wo, bo):
    from concourse.bass_utils import run_bass_kernel_spmd

    Wq = np.asarray(Wq, np.float32)
    Wk = np.asarray(Wk, np.float32)
    Wv = np.asarray(Wv, np.float32)
    Wo = np.asarray(Wo, np.float32)
    bq = np.asarray(bq, np.float32)
    bv = np.asarray(bv, np.float32)
    bo = np.asarray(bo, np.float32)

    nc = _get_nc()
    in_maps = _make_in_maps(query_input, Wq, bq, Wk, Wv, Wo)
    res = run_bass_kernel_spmd(nc, in_maps, core_ids=list(range(NCORES)))

    gpc = NCORES // B  # groups per batch
    out = np.zeros((B, S, D), np.float32)
    for core in range(NCORES):
        b = core // gpc
        out[b] += res.results[core]["out_p"]
    # bv correction (exact) + bo, applied once on the full output
    out += (bv @ Wo + bo)[None, None, :]
    return out


# revision 46
# speedup vs baseline: 1.0363x; 1.0052x over previous
"""Self-contained 8-core Trainium2 Bass kernel for MultiHeadAttention.

Problem: B=2, S=2048, D=1024, H=16 heads (hd=64), f32, self-attention
(no mask), eval mode (dropout = identity).

Sharding: data-parallel over B (2) x tensor-parallel over heads (4 groups
of 4 heads) = 8 cores. Each core computes, for its batch b and its 4
heads: Q/K/V projections (column-sliced), attention, and a partial
output projection (row-sliced Wo). Host sums the 4 partials per batch
and adds the (bv @ Wo + bo) correction (bv never enters the kernel:
ctx rows sum probs to 1, so (ctx+bv) @ Wo = ctx @ Wo + bv @ Wo).

Algebraic simplifications used (exact):
  - bk dropped: softmax over k is invariant to the per-q constant Q.bk.
  - softmax computed without max subtraction (scores bounded ~|s|<10,
    exp is safe in f32).
  - bq folded into Q^T as a per-partition bias.
  - row normalization deferred past the P@V matmul (scale ctx instead
    of probs); row sums obtained free via an appended ones-column in V.

Performance design (v3):
  - all matmul operands bf16 (f32r streams at 0.5 col/cycle, bf16 at
    1 col/cycle); PSUM accumulation stays f32. Softmax numerator and
    denominator share the bf16 exp values so normalization error
    largely cancels.
  - scores per head-pair as two concurrent K=64 row-tiled matmuls
    (tile_position (0,0)/(64,0)).
  - ACT exp (128 x [128,1024] tiles ~ 142us) is the bottleneck, so the
    emission is software-pipelined around the scores->exp stream: PV
    matmuls, projections, normalizations and the output projection are
    drained from a pending queue in the PE-slack of each exp step.
  - PSUM budget (8 banks): sreg [128,1024] x2 = 4, ctx/outproj shared
    ring [128,512] x3 = 3, proj ring [128,512] x1 = 1.
  - host pre-arranges xt/weights so every DMA is a contiguous
    per-partition block (9 DMAs total); PE warmup matmuls + ACT table
    preload run during the DMA fill.
"""

import sys

sys.path.insert(0, "/opt/trn_rl_repo")

import numpy as np

B, S, D, H, HD = 2, 2048, 1024, 16, 64
HPC = 4  # heads per core
NCORES = 8
DC = D // 128  # 8 contraction chunks
ST = S // 128  # 16 s-tiles
QCW = 512  # q chunk width
QC = S // QCW  # 4 q chunks
KT = S // 128  # 16 k tiles

_CACHE = {}


def _build(repeat=1, ep_bufs=34, target=1.04):
    from collections import deque

    import concourse.bass as bass  # noqa: F401
    import concourse.mybir as mybir
    import concourse.tile as tile
    from concourse import bacc
    from concourse.library_config import attn as attn_lib

    F32 = mybir.dt.float32
    BF16 = mybir.dt.bfloat16
    AF = mybir.ActivationFunctionType

    nc = bacc.Bacc("TRN2", target_bir_lowering=False, debug=False)

    # host pre-arranged inputs (see _make_in_maps), one tensor per DMA,
    # ordered by when the pipeline first needs them (bq rides with wq)
    WSZ = DC * HPC * HD  # 2048 elems/partition per weight matrix
    XSZ = DC * QCW       # 4096 elems/partition per xt quarter
    wk_d = nc.dram_tensor("wka", [128, WSZ], BF16, kind="ExternalInput")
    xq_d = [nc.dram_tensor(f"xtq{i}", [128, XSZ], BF16, kind="ExternalInput")
            for i in range(QC)]
    wqb_d = nc.dram_tensor("wqb", [128, WSZ + 2], BF16, kind="ExternalInput")
    wv_d = nc.dram_tensor("wva", [128, WSZ], BF16, kind="ExternalInput")
    wo_d = nc.dram_tensor("woa", [128, WSZ], BF16, kind="ExternalInput")
    out_d = nc.dram_tensor("out_p", [S, D], F32, kind="ExternalOutput")

    with tile.TileContext(nc) as tc:
        nc.gpsimd.load_library(attn_lib)
        with (
            tc.tile_pool(name="wp", bufs=1) as wp,
            tc.tile_pool(name="xp", bufs=1) as xp,
            tc.tile_pool(name="qk", bufs=1) as qk,
            tc.tile_pool(name="vp", bufs=1) as vp,
            tc.tile_pool(name="ep", bufs=ep_bufs) as ep,
            tc.tile_pool(name="cp", bufs=1) as cp,
            tc.tile_pool(name="mp", bufs=2) as mp,
            tc.tile_pool(name="op", bufs=3) as op,
            tc.tile_pool(name="pp", bufs=2, space="PSUM") as pp,
        ):
            ones_f = wp.tile([128, 64], BF16, tag="onesf")
            nc.vector.memset(ones_f[:], 1.0)
            ones_w = wp.tile([128, 512], BF16, tag="onesw")
            nc.vector.memset(ones_w[:], 1.0)
            scrap = wp.tile([128, 8], BF16, tag="scrap")

            # ---- loads: fine-grained DMAs in just-in-time order
            wk_f = wp.tile([128, WSZ], BF16, tag="wk")
            nc.sync.dma_start(wk_f[:], wk_d[:])
            xq_f = [xp.tile([128, XSZ], BF16, tag=f"xq{i}", name=f"xq{i}")
                    for i in range(QC)]
            nc.sync.dma_start(xq_f[0][:], xq_d[0][:])
            wqb_f = wp.tile([128, WSZ + 2], BF16, tag="wqb")
            nc.sync.dma_start(wqb_f[:], wqb_d[:])
            wv_f = wp.tile([128, WSZ], BF16, tag="wv")
            nc.sync.dma_start(wv_f[:], wv_d[:])
            for i in range(1, QC):
                nc.sync.dma_start(xq_f[i][:], xq_d[i][:])
            wo_f = wp.tile([128, WSZ], BF16, tag="wo")
            nc.sync.dma_start(wo_f[:], wo_d[:])

            wk_t = wk_f.rearrange("p (c n) -> p c n", n=HPC * HD)
            wq_t = wqb_f[:, 0:WSZ].rearrange("p (c n) -> p c n", n=HPC * HD)
            bq_b = wqb_f[:, WSZ:WSZ + 2]
            bq_t = wp.tile([128, 2], F32, tag="bq")
            nc.vector.tensor_copy(bq_t[:], bq_b)
            wv_t = wv_f.rearrange("p (c n) -> p c n", n=HPC * HD)
            wo_t = wo_f.rearrange("p (g n) -> p g n", g=2)
            _xtv = [x.rearrange("p (c w) -> p c w", w=QCW) for x in xq_f]

            import contextlib
            if repeat > 1:
                _engs = [mybir.EngineType.PE, mybir.EngineType.Activation,
                         mybir.EngineType.DVE, mybir.EngineType.SP,
                         mybir.EngineType.Pool]
                rep_ctx = tc.For_i(0, repeat, hint_engines=_engs, staggered_reset=True)
            else:
                rep_ctx = contextlib.nullcontext()
            with rep_ctx:
                # ---- ACT exp-table preload + PE HAM warmup during DMA fill
                nc.scalar.activation(scrap[:, 0:8], ones_f[:, 0:8], AF.Exp)
                warm_ps = pp.tile([64, 512], F32, tag="qkv", bufs=1, name="warm")
                for _w in range(18):
                    nc.tensor.matmul(warm_ps[:], ones_f[:, 0:64], ones_w[:],
                                     start=True, stop=True)

                # ---- V accumulator [s, 4*(64+1)] with ones columns
                v1_t = vp.tile([128, ST, HPC * 65], BF16, tag="v1")
                nc.vector.memset(
                    v1_t[:].rearrange("p s (h c) -> p s h c", c=65)[:, :, :, 64], 1.0)

                def v_proj(st):
                    vps = pp.tile([128, HPC * HD], F32, tag="qkv", bufs=1, name="vps")
                    for c in range(DC):
                        nc.tensor.matmul(
                            vps[:],
                            _xtv[st // 4][:, c, (st % 4) * 128:(st % 4 + 1) * 128],
                            wv_t[:, c, :],
                            start=(c == 0),
                            stop=(c == DC - 1),
                        )
                    with nc.allow_low_precision(reason="bf16 matmul operands"):
                        nc.vector.tensor_copy(
                            v1_t[:, st, :].rearrange("p (h c) -> p h c", c=65)[:, :, 0:64],
                            vps[:].rearrange("p (h c) -> p h c", c=64),
                        )

                qt_tiles = [qk.tile([128, S], BF16, tag=f"qt{p}", name=f"qt{p}") for p in range(2)]
                kt_tiles = [qk.tile([128, S], BF16, tag=f"kt{p}", name=f"kt{p}") for p in range(2)]

                _proj_ps = {}

                def _proj(w_t, pair, qcc, cs):
                    """Half of a K/Q projection (contraction chunks cs);
                    both halves share one PSUM tile."""
                    key = (w_t.name, pair, qcc)
                    if key not in _proj_ps:
                        _proj_ps[key] = pp.tile([128, QCW], F32, tag="qkv",
                                                bufs=1, name="prps")
                    prps = _proj_ps[key]
                    for c in cs:
                        nc.tensor.matmul(
                            prps[:],
                            w_t[:, c, pair * 128:(pair + 1) * 128],
                            _xtv[qcc][:, c, :],
                            start=(c == 0),
                            stop=(c == DC - 1),
                        )
                    return prps

                def kt_proj(pair, qcc, cs=range(DC)):
                    kps = _proj(wk_t, pair, qcc, cs)
                    if cs[-1] == DC - 1:
                        qs = slice(qcc * QCW, (qcc + 1) * QCW)
                        with nc.allow_low_precision(reason="bf16 score operands"):
                            nc.vector.tensor_copy(kt_tiles[pair][:, qs], kps[:])

                def qt_proj(pair, qcc, cs=range(DC)):
                    qps = _proj(wq_t, pair, qcc, cs)
                    if cs[-1] == DC - 1:
                        qs = slice(qcc * QCW, (qcc + 1) * QCW)
                        with nc.allow_low_precision(reason="bf16 score operands"):
                            nc.vector.tensor_scalar_add(
                                qt_tiles[pair][:, qs], qps[:], bq_t[:, pair:pair + 1]
                            )

                ctxt_tiles = [cp.tile([128, S], BF16, tag=f"ct{p}", name=f"ct{p}") for p in range(2)]

                # ---- software pipeline ------------------------------------
                # pending: deque of (pe_cost_us, closure) drained in PE slack
                pending = deque()

                def drain(budget):
                    while pending and budget > 0.0:
                        cost, fn = pending.popleft()
                        fn()
                        budget -= cost
                    return budget

                ctx_ps = {}   # (pair, qcc) -> [h0_tile, h1_tile]
                expt_of = {}  # (pair, qcc, r) -> expt tile

                def scores_exp(pair, qcc, r):
                    qs = slice(qcc * QCW, (qcc + 1) * QCW)
                    sreg = pp.tile([128, 2 * QCW], F32, tag="big")
                    expt = ep.tile([128, 2 * QCW], BF16, tag="exp")
                    for h in range(2):
                        nc.tensor.matmul(
                            sreg[:, h * QCW:(h + 1) * QCW],
                            kt_tiles[pair][64 * h:64 * (h + 1), r * 128:(r + 1) * 128],
                            qt_tiles[pair][64 * h:64 * (h + 1), qs],
                            start=True,
                            stop=True,
                            tile_position=(64 * h, 0),
                        )
                    with nc.allow_low_precision(reason="bf16 probs"):
                        nc.scalar.activation(expt[:], sreg[:], AF.Exp, scale=0.125)
                    expt_of[(pair, qcc, r)] = expt

                def pv(pair, qcc, h, r):
                    key = (pair, qcc)
                    if key not in ctx_ps:
                        ctx_ps[key] = [
                            pp.tile([65, QCW], F32, tag="ctx", bufs=3,
                                    name=f"ctx{pair}{qcc}{_h}")
                            for _h in range(2)
                        ]
                    hh = 2 * pair + h
                    expt = expt_of[(pair, qcc, r)]
                    nc.tensor.matmul(
                        ctx_ps[key][h][:],
                        v1_t[:, r, 65 * hh:65 * hh + 65],
                        expt[:, h * QCW:(h + 1) * QCW],
                        start=(r == 0),
                        stop=(r == KT - 1),
                    )
                    if h == 1:  # h1 trails h0, so it is the last reader
                        expt_of.pop((pair, qcc, r), None)

                def norm(pair, qcc, h, sub=0, w=QCW):
                    # normalize a w-wide slice (sub indexes units of w)
                    o = sub * w
                    qs = slice(qcc * QCW + o, qcc * QCW + o + w)
                    cps = ctx_ps[(pair, qcc)][h]
                    rsum = mp.tile([1, QCW], F32, tag="rsum")
                    nc.vector.reciprocal(rsum[:, 0:w], cps[64:65, o:o + w])
                    bct = mp.tile([64, QCW], F32, tag="bc")
                    nc.gpsimd.partition_broadcast(bct[:, 0:w], rsum[:, 0:w])
                    with nc.allow_low_precision(reason="bf16 matmul operands"):
                        nc.vector.tensor_mul(
                            ctxt_tiles[pair][64 * h:64 * (h + 1), qs],
                            cps[0:64, o:o + w],
                            bct[:, 0:w],
                        )

                osb_of = {}

                def outproj_block(qcc, sub, d2):
                    q0 = qcc * QCW + sub * 128
                    ops = pp.tile([128, 512], F32, tag="ctx", bufs=3, name="ops")
                    for pair in range(2):
                        nc.tensor.matmul(
                            ops[:],
                            ctxt_tiles[pair][:, q0:q0 + 128],
                            wo_t[:, pair, d2 * 512:(d2 + 1) * 512],
                            start=(pair == 0),
                            stop=(pair == 1),
                        )
                    key = (qcc, sub)
                    if key not in osb_of:
                        osb_of[key] = op.tile([128, 2, 512], F32, tag="osb",
                                              name="osb")
                    osb = osb_of.pop(key) if d2 == 1 else osb_of[key]
                    nc.vector.tensor_copy(osb[:, d2, :], ops[:])
                    if d2 == 1:
                        # one 4KB-per-partition DMA per 128-row output block
                        nc.sync.dma_start(
                            out_d[q0:q0 + 128, :],
                            osb[:].rearrange("p a b -> p (a b)"))

                def push_loop_work(pair, qcc):
                    """Queue all h0 PVs (h0 norms pop mid-next-loop, off the
                    critical path), then h1 PVs with norms and (pair 1) the
                    output projection interleaved per 128-sub at the end so
                    only the short h1 chain sits at the loop boundary."""
                    items = []
                    last = (pair == 1 and qcc == QC - 1)
                    # output projection of TWO qc's ago pops at the queue
                    # front: its norm completed a whole loop earlier, so it
                    # can never head-of-line-block the PE stream
                    if pair == 1 and qcc >= 1:
                        for s in range(4):
                            for d2 in range(2):
                                items.append((0.45, (lambda q=qcc - 1, s=s, d=d2:
                                                     outproj_block(q, s, d))))
                    if last:
                        # self-PVs for r < KT-2 were pushed inline during the
                        # loop; finish h0 first so its norms start on DVE
                        # while PE runs the remaining h1 PVs
                        for r in range(KT - 2, KT):
                            items.append((0.22, (lambda r=r: pv(1, QC - 1, 0, r))))
                        for s in range(4):
                            items.append((0.05, (lambda s=s: norm(1, QC - 1, 0, s, 128))))
                        for r in range(KT - 2, KT):
                            items.append((0.22, (lambda r=r: pv(1, QC - 1, 1, r))))
                        pending.extend(items)
                        return
                    for r in range(KT):
                        items.append((0.22, (lambda p=pair, q=qcc, r=r: pv(p, q, 0, r))))
                    if pair == 0:
                        items.append((0.05, (lambda q=qcc: norm(0, q, 0, 0, 256))))
                        items.append((0.05, (lambda q=qcc: norm(0, q, 0, 1, 256))))
                    else:
                        for s in range(4):
                            items.append((0.05, (lambda q=qcc, s=s:
                                                 norm(1, q, 0, s, 128))))
                    for r in range(KT):
                        items.append((0.22, (lambda p=pair, q=qcc, r=r: pv(p, q, 1, r))))
                    if pair == 0:
                        items.append((0.05, (lambda q=qcc: norm(0, q, 1, 0, 256))))
                        items.append((0.05, (lambda q=qcc: norm(0, q, 1, 1, 256))))
                    else:
                        for s in range(4):
                            items.append((0.05, (lambda q=qcc, s=s:
                                                 norm(1, q, 1, s, 128))))
                    pending.extend(items)

                # hard injections: (loop_index, r) -> list of (cost, fn)
                hard = {}

                def add_hard(li, r, cost, fn):
                    hard.setdefault((li, r), []).append((cost, fn))

                H1, H2 = range(0, DC // 2), range(DC // 2, DC)

                # qt for next qc of same pair, split in two halves
                for li, (pair, qcc) in enumerate(
                        [(p, q) for p in range(2) for q in range(QC)]):
                    if qcc < QC - 1:
                        add_hard(li, 9, 0.85, (lambda p=pair, q=qcc + 1: qt_proj(p, q, H1)))
                        add_hard(li, 11, 0.85, (lambda p=pair, q=qcc + 1: qt_proj(p, q, H2)))
                # kt/qt for pair 1 spread over pair-0 loops 2,3
                add_hard(2, 2, 0.85, lambda: kt_proj(1, 0, H1))
                add_hard(2, 4, 0.85, lambda: kt_proj(1, 0, H2))
                add_hard(2, 6, 0.85, lambda: kt_proj(1, 1, H1))
                add_hard(2, 8, 0.85, lambda: kt_proj(1, 1, H2))
                add_hard(3, 2, 0.85, lambda: kt_proj(1, 2, H1))
                add_hard(3, 4, 0.85, lambda: kt_proj(1, 2, H2))
                add_hard(3, 6, 0.85, lambda: kt_proj(1, 3, H1))
                add_hard(3, 8, 0.85, lambda: kt_proj(1, 3, H2))
                add_hard(3, 13, 0.85, lambda: qt_proj(1, 0, H1))
                add_hard(3, 14, 0.85, lambda: qt_proj(1, 0, H2))
                # loop 0: V projection + JIT kt(0,1..3) as the xt DMA lands
                vq = 0
                for r in range(KT):
                    if r in (3, 7, 11):
                        add_hard(0, r, 1.7, (lambda q=r // 4 + 1: kt_proj(0, q)))
                    else:
                        add_hard(0, r, 0.86, (lambda st=vq: v_proj(st)))
                        vq += 1
                for j in range(3):
                    add_hard(1, j, 0.86, (lambda st=13 + j: v_proj(st)))

                # ---- prelude
                kt_proj(0, 0)
                qt_proj(0, 0)

                # ---- main loops
                for li, (pair, qcc) in enumerate(
                        [(p, q) for p in range(2) for q in range(QC)]):
                    for r in range(KT):
                        budget = target - 0.21
                        for cost, fn in hard.pop((li, r), []):
                            fn()
                            budget -= cost
                        scores_exp(pair, qcc, r)
                        if li == 7 and r >= 2:
                            # last loop: self-PVs join the queue right away
                            # so they drain in-loop instead of in the tail
                            pending.append((0.22, (lambda r=r - 2: pv(1, QC - 1, 0, r))))
                            pending.append((0.22, (lambda r=r - 2: pv(1, QC - 1, 1, r))))
                        drain(budget)
                    push_loop_work(pair, qcc)

                # ---- drain tail; last loop's h1 norms split per 128-wide
                # sub-chunk, pipelined with its output projection
                while pending:
                    _, fn = pending.popleft()
                    fn()
                for sub in range(4):
                    norm(1, QC - 1, 1, sub=sub, w=128)
                    outproj_block(QC - 1, sub, 0)
                    outproj_block(QC - 1, sub, 1)

    nc.compile()
    return nc


def _get_nc(repeat=1):
    key = repeat
    if key not in _CACHE:
        _CACHE[key] = _build(repeat)
    return _CACHE[key]


def _part_major_flat(a):
    """[G*128, N] -> [128, G*N] (partition-major, flattened)."""
    n = a.shape[1]
    return a.reshape(-1, 128, n).transpose(1, 0, 2).reshape(128, -1)


def _make_in_maps(query_input, Wq, bq, Wk, Wv, Wo):
    import ml_dtypes

    BF = ml_dtypes.bfloat16
    x = np.asarray(query_input, dtype=np.float32)
    in_maps = []
    for core in range(NCORES):
        b, g = divmod(core, NCORES // B)
        cs = slice(g * HPC * HD, (g + 1) * HPC * HD)
        xt = x[b].T.astype(BF)  # [D, S]
        # [D, S] -> [128, QC, DC*QCW]: partition p, quarter qc, chunk c
        xtq = xt.reshape(DC, 128, QC, QCW).transpose(1, 2, 0, 3).reshape(128, QC, -1)
        wqa = _part_major_flat(Wq[:, cs].astype(BF))
        bq2 = bq[cs].reshape(2, 128).T.astype(BF)
        m = {
            "wka": _part_major_flat(Wk[:, cs].astype(BF)),
            "wqb": np.concatenate([wqa, bq2], axis=1),
            "wva": _part_major_flat(Wv[:, cs].astype(BF)),
            "woa": _part_major_flat(Wo[cs, :].astype(BF)),
        }
        for i in range(QC):
            m[f"xtq{i}"] = xtq[:, i]
        in_maps.append({k: np.ascontiguousarray(v) for k, v in m.items()})
    return in_maps


def kernel(query_input, Wq, bq, Wk, bk, Wv, bv, Wo, bo):
    from concourse.bass_utils import run_bass_kernel_spmd

    Wq = np.asarray(Wq, np.float32)
    Wk = np.asarray(Wk, np.float32)
    Wv = np.asarray(Wv, np.float32)
    Wo = np.asarray(Wo, np.float32)
    bq = np.asarray(bq, np.float32)
    bv = np.asarray(bv, np.float32)
    bo = np.asarray(bo, np.float32)

    nc = _get_nc()
    in_maps = _make_in_maps(query_input, Wq, bq, Wk, Wv, Wo)
    res = run_bass_kernel_spmd(nc, in_maps, core_ids=list(range(NCORES)))

    gpc = NCORES // B  # groups per batch
    out = np.zeros((B, S, D), np.float32)
    for core in range(NCORES):
        b = core // gpc
        out[b] += res.results[core]["out_p"]
    # bv correction (exact) + bo, applied once on the full output
    out += (bv @ Wo + bo)[None, None, :]
    return out


# revision 50
# speedup vs baseline: 1.0473x; 1.0106x over previous
"""Self-contained 8-core Trainium2 Bass kernel for MultiHeadAttention.

Problem: B=2, S=2048, D=1024, H=16 heads (hd=64), f32, self-attention
(no mask), eval mode (dropout = identity).

Sharding: data-parallel over B (2) x tensor-parallel over heads (4 groups
of 4 heads) = 8 cores. Each core computes, for its batch b and its 4
heads: Q/K/V projections (column-sliced), attention, and a partial
output projection (row-sliced Wo). Host sums the 4 partials per batch
and adds the (bv @ Wo + bo) correction (bv never enters the kernel:
ctx rows sum probs to 1, so (ctx+bv) @ Wo = ctx @ Wo + bv @ Wo).

Algebraic simplifications used (exact):
  - bk dropped: softmax over k is invariant to the per-q constant Q.bk.
  - softmax computed without max subtraction (scores bounded ~|s|<10,
    exp is safe in f32).
  - bq folded into Q^T as a per-partition bias.
  - row normalization deferred past the P@V matmul (scale ctx instead
    of probs); row sums obtained free via an appended ones-column in V.

Performance design (v3):
  - all matmul operands bf16 (f32r streams at 0.5 col/cycle, bf16 at
    1 col/cycle); PSUM accumulation stays f32. Softmax numerator and
    denominator share the bf16 exp values so normalization error
    largely cancels.
  - scores per head-pair as two concurrent K=64 row-tiled matmuls
    (tile_position (0,0)/(64,0)).
  - ACT exp (128 x [128,1024] tiles ~ 142us) is the bottleneck, so the
    emission is software-pipelined around the scores->exp stream: PV
    matmuls, projections, normalizations and the output projection are
    drained from a pending queue in the PE-slack of each exp step.
  - PSUM budget (8 banks): sreg [128,1024] x2 = 4, ctx/outproj shared
    ring [128,512] x3 = 3, proj ring [128,512] x1 = 1.
  - host pre-arranges xt/weights so every DMA is a contiguous
    per-partition block (9 DMAs total); PE warmup matmuls + ACT table
    preload run during the DMA fill.
"""

import sys

sys.path.insert(0, "/opt/trn_rl_repo")

import numpy as np

B, S, D, H, HD = 2, 2048, 1024, 16, 64
HPC = 4  # heads per core
NCORES = 8
DC = D // 128  # 8 contraction chunks
ST = S // 128  # 16 s-tiles
QCW = 512  # q chunk width
QC = S // QCW  # 4 q chunks
KT = S // 128  # 16 k tiles

_CACHE = {}


def _build(repeat=1, ep_bufs=34, target=1.04):
    from collections import deque

    import concourse.bass as bass  # noqa: F401
    import concourse.mybir as mybir
    import concourse.tile as tile
    from concourse import bacc
    from concourse.library_config import attn as attn_lib

    F32 = mybir.dt.float32
    BF16 = mybir.dt.bfloat16
    AF = mybir.ActivationFunctionType

    nc = bacc.Bacc("TRN2", target_bir_lowering=False, debug=False)

    # host pre-arranged inputs (see _make_in_maps), one tensor per DMA,
    # ordered by when the pipeline first needs them (bq rides with wq)
    WSZ = DC * HPC * HD  # 2048 elems/partition per weight matrix
    XSZ = DC * QCW       # 4096 elems/partition per xt quarter
    wk_d = nc.dram_tensor("wka", [128, WSZ], BF16, kind="ExternalInput")
    xq_d = [nc.dram_tensor(f"xtq{i}", [128, XSZ], BF16, kind="ExternalInput")
            for i in range(QC)]
    wqb_d = nc.dram_tensor("wqb", [128, WSZ + 2], BF16, kind="ExternalInput")
    wv_d = nc.dram_tensor("wva", [128, WSZ], BF16, kind="ExternalInput")
    wo_d = nc.dram_tensor("woa", [128, WSZ], BF16, kind="ExternalInput")
    out_d = nc.dram_tensor("out_p", [S, D], F32, kind="ExternalOutput")

    with tile.TileContext(nc) as tc:
        nc.gpsimd.load_library(attn_lib)
        with (
            tc.tile_pool(name="wp", bufs=1) as wp,
            tc.tile_pool(name="xp", bufs=1) as xp,
            tc.tile_pool(name="qk", bufs=1) as qk,
            tc.tile_pool(name="vp", bufs=1) as vp,
            tc.tile_pool(name="ep", bufs=ep_bufs) as ep,
            tc.tile_pool(name="cp", bufs=1) as cp,
            tc.tile_pool(name="mp", bufs=2) as mp,
            tc.tile_pool(name="op", bufs=3) as op,
            tc.tile_pool(name="pp", bufs=2, space="PSUM") as pp,
        ):
            ones_f = wp.tile([128, 64], BF16, tag="onesf")
            nc.vector.memset(ones_f[:], 1.0)
            ones_w = wp.tile([128, 512], BF16, tag="onesw")
            nc.vector.memset(ones_w[:], 1.0)
            scrap = wp.tile([128, 8], BF16, tag="scrap")

            # ---- loads: fine-grained DMAs in just-in-time order
            wk_f = wp.tile([128, WSZ], BF16, tag="wk")
            nc.sync.dma_start(wk_f[:], wk_d[:])
            xq_f = [xp.tile([128, XSZ], BF16, tag=f"xq{i}", name=f"xq{i}")
                    for i in range(QC)]
            nc.sync.dma_start(xq_f[0][:], xq_d[0][:])
            wqb_f = wp.tile([128, WSZ + 2], BF16, tag="wqb")
            nc.sync.dma_start(wqb_f[:], wqb_d[:])
            wv_f = wp.tile([128, WSZ], BF16, tag="wv")
            nc.sync.dma_start(wv_f[:], wv_d[:])
            for i in range(1, QC):
                nc.sync.dma_start(xq_f[i][:], xq_d[i][:])
            wo_f = wp.tile([128, WSZ], BF16, tag="wo")
            nc.sync.dma_start(wo_f[:], wo_d[:])

            wk_t = wk_f.rearrange("p (c n) -> p c n", n=HPC * HD)
            wq_t = wqb_f[:, 0:WSZ].rearrange("p (c n) -> p c n", n=HPC * HD)
            bq_b = wqb_f[:, WSZ:WSZ + 2]
            bq_t = wp.tile([128, 2], F32, tag="bq")
            nc.vector.tensor_copy(bq_t[:], bq_b)
            wv_t = wv_f.rearrange("p (c n) -> p c n", n=HPC * HD)
            wo_t = wo_f.rearrange("p (g n) -> p g n", g=2)
            _xtv = [x.rearrange("p (c w) -> p c w", w=QCW) for x in xq_f]

            import contextlib
            if repeat > 1:
                _engs = [mybir.EngineType.PE, mybir.EngineType.Activation,
                         mybir.EngineType.DVE, mybir.EngineType.SP,
                         mybir.EngineType.Pool]
                rep_ctx = tc.For_i(0, repeat, hint_engines=_engs, staggered_reset=True)
            else:
                rep_ctx = contextlib.nullcontext()
            with rep_ctx:
                # ---- ACT exp-table preload + PE HAM warmup during DMA fill
                nc.scalar.activation(scrap[:, 0:8], ones_f[:, 0:8], AF.Exp)
                warm_ps = pp.tile([64, 512], F32, tag="qkv", bufs=1, name="warm")
                for _w in range(18):
                    nc.tensor.matmul(warm_ps[:], ones_f[:, 0:64], ones_w[:],
                                     start=True, stop=True)

                # ---- V accumulator [s, 4*(64+1)] with ones columns
                v1_t = vp.tile([128, ST, HPC * 65], BF16, tag="v1")
                nc.vector.memset(
                    v1_t[:].rearrange("p s (h c) -> p s h c", c=65)[:, :, :, 64], 1.0)

                def v_proj(st):
                    vps = pp.tile([128, HPC * HD], F32, tag="qkv", bufs=1, name="vps")
                    for c in range(DC):
                        nc.tensor.matmul(
                            vps[:],
                            _xtv[st // 4][:, c, (st % 4) * 128:(st % 4 + 1) * 128],
                            wv_t[:, c, :],
                            start=(c == 0),
                            stop=(c == DC - 1),
                        )
                    with nc.allow_low_precision(reason="bf16 matmul operands"):
                        nc.vector.tensor_copy(
                            v1_t[:, st, :].rearrange("p (h c) -> p h c", c=65)[:, :, 0:64],
                            vps[:].rearrange("p (h c) -> p h c", c=64),
                        )

                qt_tiles = [qk.tile([128, S], BF16, tag=f"qt{p}", name=f"qt{p}") for p in range(2)]
                kt_tiles = [qk.tile([128, S], BF16, tag=f"kt{p}", name=f"kt{p}") for p in range(2)]

                _proj_ps = {}

                def _proj(w_t, pair, qcc, cs):
                    """Half of a K/Q projection (contraction chunks cs);
                    both halves share one PSUM tile."""
                    key = (w_t.name, pair, qcc)
                    if key not in _proj_ps:
                        _proj_ps[key] = pp.tile([128, QCW], F32, tag="qkv",
                                                bufs=1, name="prps")
                    prps = _proj_ps[key]
                    for c in cs:
                        nc.tensor.matmul(
                            prps[:],
                            w_t[:, c, pair * 128:(pair + 1) * 128],
                            _xtv[qcc][:, c, :],
                            start=(c == 0),
                            stop=(c == DC - 1),
                        )
                    return prps

                def kt_proj(pair, qcc, cs=range(DC)):
                    kps = _proj(wk_t, pair, qcc, cs)
                    if cs[-1] == DC - 1:
                        qs = slice(qcc * QCW, (qcc + 1) * QCW)
                        with nc.allow_low_precision(reason="bf16 score operands"):
                            nc.vector.tensor_copy(kt_tiles[pair][:, qs], kps[:])

                def qt_proj(pair, qcc, cs=range(DC)):
                    qps = _proj(wq_t, pair, qcc, cs)
                    if cs[-1] == DC - 1:
                        qs = slice(qcc * QCW, (qcc + 1) * QCW)
                        with nc.allow_low_precision(reason="bf16 score operands"):
                            nc.vector.tensor_scalar_add(
                                qt_tiles[pair][:, qs], qps[:], bq_t[:, pair:pair + 1]
                            )

                ctxt_tiles = [cp.tile([128, S], BF16, tag=f"ct{p}", name=f"ct{p}") for p in range(2)]

                # ---- software pipeline ------------------------------------
                # pending: deque of (pe_cost_us, closure) drained in PE slack
                pending = deque()

                def drain(budget):
                    while pending and budget > 0.0:
                        cost, fn = pending.popleft()
                        fn()
                        budget -= cost
                    return budget

                ctx_ps = {}   # (pair, qcc) -> [h0_tile, h1_tile]
                expt_of = {}  # (pair, qcc, r) -> expt tile

                def scores_exp(pair, qcc, r):
                    qs = slice(qcc * QCW, (qcc + 1) * QCW)
                    sreg = pp.tile([128, 2 * QCW], F32, tag="big")
                    expt = ep.tile([128, 2 * QCW], BF16, tag="exp")
                    for h in range(2):
                        nc.tensor.matmul(
                            sreg[:, h * QCW:(h + 1) * QCW],
                            kt_tiles[pair][64 * h:64 * (h + 1), r * 128:(r + 1) * 128],
                            qt_tiles[pair][64 * h:64 * (h + 1), qs],
                            start=True,
                            stop=True,
                            tile_position=(64 * h, 0),
                        )
                    with nc.allow_low_precision(reason="bf16 probs"):
                        nc.scalar.activation(expt[:], sreg[:], AF.Exp, scale=0.125)
                    expt_of[(pair, qcc, r)] = expt

                def pv(pair, qcc, h, r):
                    key = (pair, qcc)
                    if key not in ctx_ps:
                        ctx_ps[key] = [
                            pp.tile([65, QCW], F32, tag="ctx", bufs=3,
                                    name=f"ctx{pair}{qcc}{_h}")
                            for _h in range(2)
                        ]
                    hh = 2 * pair + h
                    expt = expt_of[(pair, qcc, r)]
                    nc.tensor.matmul(
                        ctx_ps[key][h][:],
                        v1_t[:, r, 65 * hh:65 * hh + 65],
                        expt[:, h * QCW:(h + 1) * QCW],
                        start=(r == 0),
                        stop=(r == KT - 1),
                    )
                    if h == 1:  # h1 trails h0, so it is the last reader
                        expt_of.pop((pair, qcc, r), None)

                def norm(pair, qcc, h, sub=0, w=QCW):
                    # normalize a w-wide slice (sub indexes units of w)
                    o = sub * w
                    qs = slice(qcc * QCW + o, qcc * QCW + o + w)
                    cps = ctx_ps[(pair, qcc)][h]
                    rsum = mp.tile([1, QCW], F32, tag="rsum")
                    nc.vector.reciprocal(rsum[:, 0:w], cps[64:65, o:o + w])
                    bct = mp.tile([64, QCW], F32, tag="bc")
                    nc.gpsimd.partition_broadcast(bct[:, 0:w], rsum[:, 0:w])
                    with nc.allow_low_precision(reason="bf16 matmul operands"):
                        nc.vector.tensor_mul(
                            ctxt_tiles[pair][64 * h:64 * (h + 1), qs],
                            cps[0:64, o:o + w],
                            bct[:, 0:w],
                        )

                osb_of = {}

                def outproj_block(qcc, sub, d2):
                    q0 = qcc * QCW + sub * 128
                    ops = pp.tile([128, 512], F32, tag="ctx", bufs=3, name="ops")
                    for pair in range(2):
                        nc.tensor.matmul(
                            ops[:],
                            ctxt_tiles[pair][:, q0:q0 + 128],
                            wo_t[:, pair, d2 * 512:(d2 + 1) * 512],
                            start=(pair == 0),
                            stop=(pair == 1),
                        )
                    key = (qcc, sub)
                    if key not in osb_of:
                        osb_of[key] = op.tile([128, 2, 512], F32, tag="osb",
                                              name="osb")
                    osb = osb_of.pop(key) if d2 == 1 else osb_of[key]
                    nc.vector.tensor_copy(osb[:, d2, :], ops[:])
                    if d2 == 1:
                        # one 4KB-per-partition DMA per 128-row output block
                        nc.sync.dma_start(
                            out_d[q0:q0 + 128, :],
                            osb[:].rearrange("p a b -> p (a b)"))

                def push_loop_work(pair, qcc):
                    """Queue all h0 PVs (h0 norms pop mid-next-loop, off the
                    critical path), then h1 PVs with norms and (pair 1) the
                    output projection interleaved per 128-sub at the end so
                    only the short h1 chain sits at the loop boundary."""
                    items = []
                    last = (pair == 1 and qcc == QC - 1)
                    if last:
                        # self-PVs for r < KT-2 were pushed inline during the
                        # loop; finish h0 first so its norms start on DVE
                        # while PE runs the remaining h1 PVs
                        for r in range(KT - 2, KT):
                            items.append((0.22, (lambda r=r: pv(1, QC - 1, 0, r))))
                        for s in range(4):
                            items.append((0.05, (lambda s=s: norm(1, QC - 1, 0, s, 128))))
                        for r in range(KT - 2, KT):
                            items.append((0.22, (lambda r=r: pv(1, QC - 1, 1, r))))
                        pending.extend(items)
                        return
                    for r in range(KT):
                        items.append((0.22, (lambda p=pair, q=qcc, r=r: pv(p, q, 0, r))))
                    if pair == 0:
                        items.append((0.05, (lambda q=qcc: norm(0, q, 0, 0, 256))))
                        items.append((0.05, (lambda q=qcc: norm(0, q, 0, 1, 256))))
                    else:
                        for s in range(4):
                            items.append((0.05, (lambda q=qcc, s=s:
                                                 norm(1, q, 0, s, 128))))
                    for r in range(KT):
                        items.append((0.22, (lambda p=pair, q=qcc, r=r: pv(p, q, 1, r))))
                    if pair == 0:
                        items.append((0.05, (lambda q=qcc: norm(0, q, 1, 0, 256))))
                        items.append((0.05, (lambda q=qcc: norm(0, q, 1, 1, 256))))
                    else:
                        for s in range(4):
                            items.append((0.05, (lambda q=qcc, s=s:
                                                 norm(1, q, 1, s, 128))))
                            items.append((0.45, (lambda q=qcc, s=s:
                                                 outproj_block(q, s, 0))))
                            items.append((0.45, (lambda q=qcc, s=s:
                                                 outproj_block(q, s, 1))))
                    pending.extend(items)

                # hard injections: (loop_index, r) -> list of (cost, fn)
                hard = {}

                def add_hard(li, r, cost, fn):
                    hard.setdefault((li, r), []).append((cost, fn))

                H1, H2 = range(0, DC // 2), range(DC // 2, DC)

                # qt for next qc of same pair, split in two halves
                for li, (pair, qcc) in enumerate(
                        [(p, q) for p in range(2) for q in range(QC)]):
                    if qcc < QC - 1:
                        add_hard(li, 9, 0.85, (lambda p=pair, q=qcc + 1: qt_proj(p, q, H1)))
                        add_hard(li, 11, 0.85, (lambda p=pair, q=qcc + 1: qt_proj(p, q, H2)))
                # kt/qt for pair 1 spread over pair-0 loops 2,3
                add_hard(2, 2, 0.85, lambda: kt_proj(1, 0, H1))
                add_hard(2, 4, 0.85, lambda: kt_proj(1, 0, H2))
                add_hard(2, 6, 0.85, lambda: kt_proj(1, 1, H1))
                add_hard(2, 8, 0.85, lambda: kt_proj(1, 1, H2))
                add_hard(3, 2, 0.85, lambda: kt_proj(1, 2, H1))
                add_hard(3, 4, 0.85, lambda: kt_proj(1, 2, H2))
                add_hard(3, 6, 0.85, lambda: kt_proj(1, 3, H1))
                add_hard(3, 8, 0.85, lambda: kt_proj(1, 3, H2))
                add_hard(3, 13, 0.85, lambda: qt_proj(1, 0, H1))
                add_hard(3, 14, 0.85, lambda: qt_proj(1, 0, H2))
                # loop 0: V projection + JIT kt(0,1..3) as the xt DMA lands
                vq = 0
                for r in range(KT):
                    if r in (3, 7, 11):
                        add_hard(0, r, 1.7, (lambda q=r // 4 + 1: kt_proj(0, q)))
                    else:
                        add_hard(0, r, 0.86, (lambda st=vq: v_proj(st)))
                        vq += 1
                for j in range(3):
                    add_hard(1, j, 0.86, (lambda st=13 + j: v_proj(st)))

                # ---- prelude
                kt_proj(0, 0)
                qt_proj(0, 0)

                # ---- main loops
                for li, (pair, qcc) in enumerate(
                        [(p, q) for p in range(2) for q in range(QC)]):
                    for r in range(KT):
                        budget = target - 0.21
                        for cost, fn in hard.pop((li, r), []):
                            fn()
                            budget -= cost
                        scores_exp(pair, qcc, r)
                        if li == 7 and r >= 2:
                            # last loop: self-PVs join the queue right away
                            # so they drain in-loop instead of in the tail
                            pending.append((0.22, (lambda r=r - 2: pv(1, QC - 1, 0, r))))
                            pending.append((0.22, (lambda r=r - 2: pv(1, QC - 1, 1, r))))
                        drain(budget)
                    push_loop_work(pair, qcc)

                # ---- drain tail; last loop's h1 norms split per 128-wide
                # sub-chunk, pipelined with its output projection
                while pending:
                    _, fn = pending.popleft()
                    fn()
                for sub in range(4):
                    norm(1, QC - 1, 1, sub=sub, w=128)
                    outproj_block(QC - 1, sub, 0)
                    outproj_block(QC - 1, sub, 1)

    nc.compile()
    return nc


def _get_nc(repeat=1):
    key = repeat
    if key not in _CACHE:
        _CACHE[key] = _build(repeat)
    return _CACHE[key]


def _part_major_flat(a):
    """[G*128, N] -> [128, G*N] (partition-major, flattened)."""
    n = a.shape[1]
    return a.reshape(-1, 128, n).transpose(1, 0, 2).reshape(128, -1)


def _make_in_maps(query_input, Wq, bq, Wk, Wv, Wo):
    import ml_dtypes

    BF = ml_dtypes.bfloat16
    x = np.asarray(query_input, dtype=np.float32)
    in_maps = []
    for core in range(NCORES):
        b, g = divmod(core, NCORES // B)
        cs = slice(g * HPC * HD, (g + 1) * HPC * HD)
        xt = x[b].T.astype(BF)  # [D, S]
        # [D, S] -> [128, QC, DC*QCW]: partition p, quarter qc, chunk c
        xtq = xt.reshape(DC, 128, QC, QCW).transpose(1, 2, 0, 3).reshape(128, QC, -1)
        wqa = _part_major_flat(Wq[:, cs].astype(BF))
        bq2 = bq[cs].reshape(2, 128).T.astype(BF)
        m = {
            "wka": _part_major_flat(Wk[:, cs].astype(BF)),
            "wqb": np.concatenate([wqa, bq2], axis=1),
            "wva": _part_major_flat(Wv[:, cs].astype(BF)),
            "woa": _part_major_flat(Wo[cs, :].astype(BF)),
        }
        for i in range(QC):
            m[f"xtq{i}"] = xtq[:, i]
        in_maps.append({k: np.ascontiguousarray(v) for k, v in m.items()})
    return in_maps


def kernel(query_input, Wq, bq, Wk, bk, Wv, bv, Wo, bo):
    from concourse.bass_utils import run_bass_kernel_spmd

    Wq = np.asarray(Wq, np.float32)
    Wk = np.asarray(Wk, np.float32)
    Wv = np.asarray(Wv, np.float32)
    Wo = np.asarray(Wo, np.float32)
    bq = np.asarray(bq, np.float32)
    bv = np.asarray(bv, np.float32)
    bo = np.asarray(bo, np.float32)

    nc = _get_nc()
    in_maps = _make_in_maps(query_input, Wq, bq, Wk, Wv, Wo)
    res = run_bass_kernel_spmd(nc, in_maps, core_ids=list(range(NCORES)))

    gpc = NCORES // B  # groups per batch
    out = np.zeros((B, S, D), np.float32)
    for core in range(NCORES):
        b = core // gpc
        out[b] += res.results[core]["out_p"]
    # bv correction (exact) + bo, applied once on the full output
    out += (bv @ Wo + bo)[None, None, :]
    return out
